# revision 33
# baseline (speedup 1.0000x reference)
"""BarrierNet forward pass on 8 Trainium2 NeuronCores (pure data parallel).

Network (per sample, batch 8192 sharded 1024/core):
  x[5] -> 1024 -> 1024 -> {512, 512} -> {512, 512} -> two 2-wide heads
  followed by a closed-form single-constraint QP projection (dCBF barrier).

Layout strategy per core:
  - L1 runs f32r (x precision matters, layer is tiny); all heavy layers
    (L2, L31/32, L41/42, heads) run fp8 e4m3 with DoubleRow perf mode:
    2 contraction rows per PE cell -> half the matmul instructions.
    End-to-end quantization error measured at ~2e-3 norm rel (tol 2e-2).
  - Activations are stored as PAIRED tiles [128, 2, BT] fp8 so each
    DoubleRow matmul consumes k-tiles (2j, 2j+1) from one SBUF tile.
  - A short stream of dummy bf16 matmuls runs while input DMAs land,
    so the PE HAM clock-gate is warm (2.4 GHz) when real work starts.
  - Each 512-sample batch tile runs the full pipeline so the tile-0
    epilogue (DVE/ACT) overlaps tile-1 matmuls on the PE.
  - Head rows land in [32, 512] staging tiles; one DVE 32x32 stream-
    transpose per tile flips them to batch-on-partition.
  - The QP/barrier epilogue runs on [32, 16] strided views (DVE/ACT/
    GpSimd elementwise ops).
  - Host does the cheap reshapes + fp8 weight quantization.
"""

import numpy as np

import concourse.bass as bass
import concourse.tile as tile
from concourse import bacc, mybir
from concourse.bass_utils import run_bass_kernel_spmd

N_CORES = 8
B_FULL = 8192
BC = B_FULL // N_CORES      # batch per core
BT = 512                    # batch tile (matmul moving free dim)
NBT = BC // BT              # batch tiles per core
GPB = BT // 32              # 32-sample groups per batch tile (16)

D1, D2, D3, D4 = 1024, 1024, 512, 512
L1C, L2C, OBS_X, OBS_Y, RADIUS = 3.0, 3.0, 0.0, 7.0, 4.0

N_WARMUP = 4                # dummy PE warmup matmuls (HAM clock-gate)

F32 = mybir.dt.float32
F32R = mybir.dt.float32r
FP8 = mybir.dt.float8e4
BF16 = mybir.dt.bfloat16
AF = mybir.ActivationFunctionType
AL = mybir.AluOpType
DR = mybir.MatmulPerfMode.DoubleRow


def build_program(consts):
    """Build the SPMD Bass program.
    consts = (mean[5], std[5], ml[2], sl[2], b51[2], b52[2])."""
    mean, std, ml, sl, b51v, b52v = consts

    nc = bacc.Bacc("TRN2", target_bir_lowering=False, debug=False,
                   num_devices=N_CORES)

    def din(name, shape, dt):
        return nc.dram_tensor(name, shape, dt, kind="ExternalInput").ap()

    # pair counts per layer (contraction k-tiles / 2)
    P2 = D1 // 256   # 4 pairs into L2
    P3 = D2 // 256   # 4 pairs into L3x
    P4 = D3 // 256   # 2 pairs into L4x
    P5 = D4 // 256   # 2 pairs into heads
    N1, N2, N3, N4 = D1 // 128, D2 // 128, D3 // 128, D4 // 128

    xT_d = din("xT", [5, BC], F32R)
    Xep_d = din("Xep", [32, NBT * GPB * 5], F32)
    W1_d = din("W1", [5, D1], F32R)
    W2_d = din("W2p", [128, P2 * 2 * D2], FP8)
    W31_d = din("W31p", [128, P3 * 2 * D3], FP8)
    W32_d = din("W32p", [128, P3 * 2 * D3], FP8)
    W41_d = din("W41p", [128, P4 * 2 * D4], FP8)
    W42_d = din("W42p", [128, P4 * 2 * D4], FP8)
    # heads zero-padded to M=128 (DoubleRow LDWEIGHTS requires wide M)
    W51_d = din("W51p", [128, P5 * 2 * 128], FP8)
    W52_d = din("W52p", [128, P5 * 2 * 128], FP8)
    b1_d = din("b1p", [128, N1], F32)
    b2_d = din("b2p", [128, N2], F32)
    b31_d = din("b31p", [128, N3], F32)
    b32_d = din("b32p", [128, N3], F32)
    b41_d = din("b41p", [128, N4], F32)
    b42_d = din("b42p", [128, N4], F32)
    b51_d = din("b51p", [2, 1], F32)
    b52_d = din("b52p", [2, 1], F32)
    out_d = nc.dram_tensor("out", [32, NBT * GPB * 2], F32,
                           kind="ExternalOutput").ap()

    with tile.TileContext(nc) as tc:
        with (
            tc.tile_pool(name="wpool", bufs=1) as wp,
            tc.tile_pool(name="acts", bufs=10) as ap_,
            tc.tile_pool(name="misc", bufs=1) as mp,
            tc.tile_pool(name="ep", bufs=1) as ep,
            tc.tile_pool(name="pmm", bufs=8, space="PSUM") as pmm,
        ):
            # ---- input/weight loads -------------------------------------
            # sync ring: L1 inputs first, then weights in need-order
            xT = mp.tile([5, BC], F32R, tag="xT", name="xT_t")
            nc.sync.dma_start(out=xT, in_=xT_d)
            w1 = wp.tile([5, D1], F32R, tag="w1", name="w1_t")
            nc.sync.dma_start(out=w1, in_=W1_d)

            def pair_w(dram, npair, ncols, nm, engine):
                """Load per-pair DoubleRow weight tiles [128, 2, ncols]."""
                ts = []
                for j in range(npair):
                    t = wp.tile([128, 2, ncols], FP8, tag=f"{nm}{j}",
                                name=f"{nm}{j}_t")
                    engine.dma_start(
                        out=t,
                        in_=dram[:, j * 2 * ncols:(j + 1) * 2 * ncols]
                        .rearrange("p (two n) -> p two n", two=2))
                    ts.append(t)
                return ts

            w2 = pair_w(W2_d, P2, D2, "w2", nc.sync)
            w31 = pair_w(W31_d, P3, D3, "w31", nc.sync)
            w32 = pair_w(W32_d, P3, D3, "w32", nc.sync)
            w41 = pair_w(W41_d, P4, D4, "w41", nc.sync)
            w42 = pair_w(W42_d, P4, D4, "w42", nc.sync)

            # gpsimd ring: small late tensors
            def gp_load(dram, shape, tg, dt=F32):
                t = mp.tile(shape, dt, tag=tg, name=f"{tg}_t")
                nc.gpsimd.dma_start(out=t, in_=dram)
                return t

            b1 = gp_load(b1_d, [128, N1], "b1")
            Xep = gp_load(Xep_d, [32, NBT * GPB * 5], "Xep")
            b2 = gp_load(b2_d, [128, N2], "b2")
            b31 = gp_load(b31_d, [128, N3], "b31")
            b32 = gp_load(b32_d, [128, N3], "b32")
            b41 = gp_load(b41_d, [128, N4], "b41")
            b42 = gp_load(b42_d, [128, N4], "b42")
            w51 = pair_w(W51_d, P5, 128, "w51", nc.gpsimd)
            w52 = pair_w(W52_d, P5, 128, "w52", nc.gpsimd)
            b51 = gp_load(b51_d, [2, 1], "b51")
            b52 = gp_load(b52_d, [2, 1], "b52")

            OUT = mp.tile([32, NBT * GPB * 2], F32, tag="OUT", name="OUT_t")

            _cbias_cache = {}

            def cbias(val, parts):
                val = float(val)
                if val not in _cbias_cache:
                    t = ep.tile([128, 1], F32, tag=f"cb{len(_cbias_cache)}",
                                name=f"cb{len(_cbias_cache)}")
                    nc.vector.memset(t, val)
                    _cbias_cache[val] = t
                return _cbias_cache[val][0:parts, :]

            def eact(out, in_, func, bias=0.0, scale=1.0):
                if isinstance(bias, float) and func not in (AF.Copy,):
                    bias = cbias(bias, in_.shape[0])
                nc.scalar.activation(out, in_, func, bias=bias, scale=scale)

            # weighted DVE/ACT alternation for relu+bias. ACT is cheaper
            # per PSUM->SBUF op (172+FD vs 120+FD but 1.2 vs 0.96 GHz) and
            # DVE also owns the epilogue: give ACT 5 of every 8.
            _rb_ctr = [0]

            def relu_bias(t, ps, bias_col):
                c = _rb_ctr[0] % 16
                _rb_ctr[0] += 1
                if c in (0, 2, 4, 7, 9, 11, 13):
                    nc.vector.tensor_scalar(t, ps, bias_col, 0.0,
                                            AL.add, AL.max)
                else:
                    nc.scalar.activation(t, ps, AF.Relu, bias=bias_col)

            HPI = float(np.pi / 2)
            PI = float(np.pi)

            def epilogue_pre(bt):
                """x-only QP/barrier quantities for batch tile bt (no head
                dependence) — runs on DVE/ACT/GpSimd while the PE is still
                in the dense layers."""
                Xv = Xep[:, bt * GPB * 5:(bt + 1) * GPB * 5] \
                    .rearrange("p (f j) -> p f j", j=5)

                def T(nm):
                    return ep.tile([32, GPB], F32, tag=nm, bufs=NBT,
                                   name=f"{nm}_b{bt}")

                def emul(o, a, b):
                    nc.vector.tensor_mul(o, a, b)

                def eadd(o, a, b):
                    nc.vector.tensor_add(o, a, b)

                def stt(o, a, s, op0, b, op1):
                    nc.vector.scalar_tensor_tensor(o, a, float(s), b, op0, op1)

                def gmul(o, a, b):
                    nc.gpsimd.tensor_mul(o, a, b)

                def gadd(o, a, b):
                    nc.gpsimd.tensor_add(o, a, b)

                def gts(o, a, s0, op0):
                    # o = (a op0 s0) + 0.0  (Pool engine lacks stt)
                    nc.gpsimd.tensor_scalar(o, a, float(s0), 0.0, op0, AL.add)

                t1r, w1r = Xv[:, :, 0], Xv[:, :, 1]
                t2r, w2r = Xv[:, :, 2], Xv[:, :, 3]

                if float(std[0]) == 1.0 and float(mean[0]) == 0.0:
                    t1m = t1r
                else:
                    t1m = T("t1m"); eact(t1m, t1r, AF.Copy, bias=float(mean[0]), scale=float(std[0]))
                if float(std[2]) == 1.0 and float(mean[2]) == 0.0:
                    t2m = t2r
                else:
                    t2m = T("t2m"); eact(t2m, t2r, AF.Copy, bias=float(mean[2]), scale=float(std[2]))

                def sincos(theta, nm):
                    ws = T(nm + "_ws"); nc.vector.add_range_wrap(ws, theta, 0.0, PI, 2 * PI)
                    s = T(nm + "_s"); eact(s, ws, AF.Sin)
                    wc = T(nm + "_wc"); nc.vector.add_range_wrap(wc, theta, HPI, PI, 2 * PI)
                    c = T(nm + "_c"); eact(c, wc, AF.Sin)
                    return s, c

                s1, c1 = sincos(t1m, "t1")
                s2, c2 = sincos(t2m, "t2")

                if float(std[1]) == 1.0 and float(mean[1]) == 0.0:
                    w1v = w1r
                else:
                    w1v = T("w1v"); eact(w1v, w1r, AF.Copy, bias=float(mean[1]), scale=float(std[1]))
                if float(std[3]) == 1.0 and float(mean[3]) == 0.0:
                    w2v = w2r
                else:
                    w2v = T("w2v"); eact(w2v, w2r, AF.Copy, bias=float(mean[3]), scale=float(std[3]))

                pxu = T("pxu"); eadd(pxu, c1, c2)
                px = T("px")
                nc.gpsimd.tensor_scalar(px, pxu, L1C, -OBS_X, AL.mult, AL.add)
                pyu = T("pyu"); eadd(pyu, s1, s2)
                py = T("py")
                nc.gpsimd.tensor_scalar(py, pyu, L1C, -OBS_Y, AL.mult, AL.add)

                a1 = T("a1"); emul(a1, s1, w1v)
                a2 = T("a2"); emul(a2, s2, w2v)
                vxn = T("vxn"); eadd(vxn, a1, a2)          # = -vx/3
                bb1 = T("bb1"); emul(bb1, c1, w1v)
                bb2 = T("bb2"); emul(bb2, c2, w2v)
                vyu = T("vyu"); eadd(vyu, bb1, bb2)
                vy = T("vy")
                nc.gpsimd.tensor_scalar(vy, vyu, 3.0, 0.0, AL.mult, AL.add)

                q1 = T("q1"); emul(q1, px, vxn)
                q2 = T("q2"); emul(q2, py, vy)
                bdot2 = T("bdot2"); stt(bdot2, q1, -3.0, AL.mult, q2, AL.add)

                # squares / Lf2b chain on GpSimd (SBUF-only ops)
                w1sq = T("w1sq"); gmul(w1sq, w1v, w1v)
                w2sq = T("w2sq"); gmul(w2sq, w2v, w2v)
                cw1 = T("cw1"); gmul(cw1, c1, w1sq)
                cw2 = T("cw2"); gmul(cw2, c2, w2sq)
                cw = T("cw"); gadd(cw, cw1, cw2)
                sw1 = T("sw1"); gmul(sw1, s1, w1sq)
                sw2 = T("sw2"); gmul(sw2, s2, w2sq)
                sw = T("sw"); gadd(sw, sw1, sw2)
                t1x = T("t1x"); gmul(t1x, px, cw)
                t2y = T("t2y"); gmul(t2y, py, sw)
                txy = T("txy"); gadd(txy, t1x, t2y)
                vv1 = T("vv1"); gmul(vv1, vxn, vxn)
                vv2 = T("vv2"); gmul(vv2, vy, vy)
                vv1s = T("vv1s"); gts(vv1s, vv1, 9.0, AL.mult)
                vv = T("vv"); gadd(vv, vv1s, vv2)
                txys = T("txys"); gts(txys, txy, -3.0, AL.mult)
                Lhalf = T("Lhalf"); gadd(Lhalf, txys, vv)

                g1a = T("g1a"); emul(g1a, px, s1)
                g1b = T("g1b"); emul(g1b, py, c1)
                g2a = T("g2a"); emul(g2a, px, s2)
                g2b = T("g2b"); emul(g2b, py, c2)
                G12 = ep.tile([32, GPB * 2], F32, tag="G12", bufs=NBT,
                              name=f"G12_b{bt}")
                G12v = G12.rearrange("p (f q) -> p f q", q=2)
                G1h, G2h = G12v[:, :, 0], G12v[:, :, 1]
                stt(G1h, g1b, -1.0, AL.mult, g1a, AL.add)  # G1/6
                stt(G2h, g2b, -1.0, AL.mult, g2a, AL.add)  # G2/6

                pxsq = T("pxsq"); gmul(pxsq, px, px)
                pysq = T("pysq"); gmul(pysq, py, py)
                psum2 = T("psum2"); gadd(psum2, pxsq, pysq)
                bar = T("bar"); gts(bar, psum2, -RADIUS * RADIUS, AL.add)

                d1 = T("d1"); emul(d1, G1h, G1h)
                d2 = T("d2"); emul(d2, G2h, G2h)
                den36 = T("den36"); stt(den36, d1, 1e-12 / 36.0, AL.add, d2, AL.add)
                nrec = T("nrec"); nc.vector.reciprocal(nrec, den36)

                return dict(bdot2=bdot2, bar=bar, Lhalf=Lhalf,
                            G1h=G1h, G2h=G2h, G12=G12, nrec=nrec)

            def epilogue_post1(bt, vtb, pre):
                """sigmoid-head-dependent part (can run before head 1)."""
                Yvb = vtb.rearrange("p (f q) -> p f q", q=32)
                sg1, sg2 = Yvb[:, :, 0], Yvb[:, :, 1]

                def T(nm):
                    return ep.tile([32, GPB], F32, tag=nm, bufs=NBT,
                                   name=f"{nm}_b{bt}")

                def emul(o, a, b):
                    nc.vector.tensor_mul(o, a, b)

                def eadd(o, a, b):
                    nc.vector.tensor_add(o, a, b)

                def stt(o, a, s, op0, b, op1):
                    nc.vector.scalar_tensor_tensor(o, a, float(s), b, op0, op1)

                bdot2, bar, Lhalf = pre["bdot2"], pre["bar"], pre["Lhalf"]

                ssum = T("ssum"); eadd(ssum, sg1, sg2)
                sprod = T("sprod"); emul(sprod, sg1, sg2)
                hb = T("hb"); emul(hb, ssum, bdot2)
                hc = T("hc"); emul(hc, sprod, bar)
                va2 = T("va2"); stt(va2, hc, 8.0, AL.mult, Lhalf, AL.add)
                va = T("va"); stt(va, hb, 4.0, AL.mult, va2, AL.add)     # h/2
                return va

            def epilogue_post2(bt, vta, pre, va):
                """head-1-dependent tail of the QP for batch tile bt."""
                Yva = vta.rearrange("p (f q) -> p f q", q=32)
                OUTv = OUT[:, bt * GPB * 2:(bt + 1) * GPB * 2] \
                    .rearrange("p (f i) -> p f i", i=2)

                def T(nm):
                    return ep.tile([32, GPB], F32, tag=nm, bufs=NBT,
                                   name=f"{nm}_b{bt}")

                def emul(o, a, b):
                    nc.vector.tensor_mul(o, a, b)

                def eadd(o, a, b):
                    nc.vector.tensor_add(o, a, b)

                def stt(o, a, s, op0, b, op1):
                    nc.vector.scalar_tensor_tensor(o, a, float(s), b, op0, op1)

                nrec = pre["nrec"]
                G12 = pre["G12"]
                P12 = Yva[:, :, 0:2]  # [32, GPB, 2]

                r12 = ep.tile([32, GPB * 2], F32, tag="r12", bufs=NBT,
                              name=f"r12_b{bt}")
                r12v = r12.rearrange("p (f q) -> p f q", q=2)
                nc.vector.tensor_mul(r12v, G12.rearrange("p (f q) -> p f q", q=2), P12)
                rs = T("rs"); eadd(rs, r12v[:, :, 0], r12v[:, :, 1])
                vb = T("vb"); stt(vb, rs, 3.0, AL.mult, va, AL.add)      # viol=-2vb

                vr = T("vr")
                nc.vector.tensor_scalar(vr, vb, -1.0, 0.0, AL.mult, AL.max)
                lam18 = T("lam18"); emul(lam18, vr, nrec)

                lam18b = bass.AP(tensor=lam18.tensor, offset=lam18.offset,
                                 ap=list(lam18.ap) + [[0, 2]])
                lg12 = ep.tile([32, GPB * 2], F32, tag="lg12", bufs=NBT,
                               name=f"lg12_b{bt}")
                lg12v = lg12.rearrange("p (f q) -> p f q", q=2)
                nc.vector.tensor_mul(
                    lg12v, lam18b, G12.rearrange("p (f q) -> p f q", q=2))
                if (float(sl[0]) == 1.0 and float(sl[1]) == 1.0
                        and float(ml[0]) == 0.0 and float(ml[1]) == 0.0):
                    # out = -(lg12/3 + P12): one DVE op straight into OUT
                    stt(OUTv[:, :, 0:2], lg12v, -1.0 / 3.0, AL.mult, P12,
                        AL.subtract)
                else:
                    u12n = ep.tile([32, GPB * 2], F32, tag="u12n", bufs=NBT,
                                   name=f"u12n_b{bt}")
                    u12v = u12n.rearrange("p (f q) -> p f q", q=2)
                    stt(u12v, lg12v, 1.0 / 3.0, AL.mult, P12, AL.add)
                    eact(OUTv[:, :, 0], u12v[:, :, 0], AF.Copy,
                         bias=-float(ml[0]) / float(sl[0]),
                         scale=-1.0 / float(sl[0]))
                    eact(OUTv[:, :, 1], u12v[:, :, 1], AF.Copy,
                         bias=-float(ml[1]) / float(sl[1]),
                         scale=-1.0 / float(sl[1]))

            def pair_tiles(nm, npair, sfx):
                return [ap_.tile([128, 2, BT], FP8, tag="act",
                                 name=f"{nm}_{j}{sfx}") for j in range(npair)]

            def layer1(bt):
                """L1 (f32r, K=5) for one batch tile -> fp8 pair tiles."""
                sfx = f"b{bt}"
                x1p = pair_tiles("x1", P2, sfx)
                rhs1 = xT[:, bt * BT:(bt + 1) * BT]
                for n in range(N1):
                    ps = pmm.tile([128, BT], F32, tag="pm", name=f"ps1_{n}{sfx}")
                    nc.tensor.matmul(ps, w1[:, n * 128:(n + 1) * 128], rhs1,
                                     start=True, stop=True)
                    relu_bias(x1p[n // 2][:, n % 2, :], ps, b1[:, n:n + 1])
                return x1p

            # head staging tiles memset up-front (GP is idle early)
            x5ab = {}
            for bt in range(NBT):
                x5a = mp.tile([32, BT], F32, tag="x5a", bufs=2,
                              name=f"x5ab{bt}")
                x5b = mp.tile([32, BT], F32, tag="x5b", bufs=2,
                              name=f"x5bb{bt}")
                nc.gpsimd.memset(x5a, 0.0)
                nc.gpsimd.memset(x5b, 0.0)
                x5ab[bt] = (x5a, x5b)

            def dense(bt, nm, xp, ws, bias, n_out, npair, engines=None):
                """fp8 DoubleRow dense layer -> fp8 pair tiles."""
                sfx = f"b{bt}"
                outp = pair_tiles(nm, n_out // 2, sfx)
                for n in range(n_out):
                    ps = pmm.tile([128, BT], F32, tag="pm",
                                  name=f"ps{nm}_{n}{sfx}")
                    for j in range(npair):
                        nc.tensor.matmul(
                            ps, ws[j][:, :, n * 128:(n + 1) * 128], xp[j],
                            start=(j == 0), stop=(j == npair - 1),
                            perf_mode=DR)
                    dst = outp[n // 2][:, n % 2, :]
                    if engines is not None:
                        if engines[n % len(engines)] == "V":
                            nc.vector.tensor_scalar(dst, ps, bias[:, n:n + 1],
                                                    0.0, AL.add, AL.max)
                        else:
                            nc.scalar.activation(dst, ps, AF.Relu,
                                                 bias=bias[:, n:n + 1])
                    else:
                        relu_bias(dst, ps, bias[:, n:n + 1])
                return outp

            def batch_tile_pipeline(bt, x1p, pre):
                """Layers 2+ for one 512-sample batch tile. The sigmoid
                branch is computed first (longest tail chain); cross-engine
                tail work (transpose, post1) is emitted late so dense relus
                aren't stuck behind it in the FIFO engine queues."""
                sfx = f"b{bt}"
                x5a, x5b = x5ab[bt]
                x2 = dense(bt, "x2", x1p, w2, b2, N2, P2)
                x32 = dense(bt, "x32", x2, w32, b32, N3, P3)
                x42 = dense(bt, "x42", x32, w42, b42, N4, P4, engines="VAVA")
                x31 = dense(bt, "x31", x2, w31, b31, N3, P3)
                ph2 = pmm.tile([128, BT], F32, tag="pm", name=f"ph2{sfx}")
                for j in range(P5):
                    nc.tensor.matmul(ph2, w52[j], x42[j],
                                     start=(j == 0), stop=(j == P5 - 1),
                                     perf_mode=DR)
                nc.scalar.activation(x5b[0:2, :], ph2[0:2, :], AF.Sigmoid,
                                     bias=b52)
                x41 = dense(bt, "x41", x31, w41, b41, N4, P4, engines="VAVA")
                vtb = mp.tile([32, BT], F32, tag="vtb", bufs=2, name=f"vtb{sfx}")
                nc.vector.transpose(vtb, x5b)
                va = epilogue_post1(bt, vtb, pre)
                ph1 = pmm.tile([128, BT], F32, tag="pm", name=f"ph1{sfx}")
                for j in range(P5):
                    nc.tensor.matmul(ph1, w51[j], x41[j],
                                     start=(j == 0), stop=(j == P5 - 1),
                                     perf_mode=DR)
                nc.scalar.activation(x5a[0:2, :], ph1[0:2, :], AF.Identity,
                                     bias=b51)
                vta = mp.tile([32, BT], F32, tag="vta", bufs=2, name=f"vta{sfx}")
                nc.vector.transpose(vta, x5a)

                epilogue_post2(bt, vta, pre, va)
                nc.sync.dma_start(
                    out=out_d[:, bt * GPB * 2:(bt + 1) * GPB * 2],
                    in_=OUT[:, bt * GPB * 2:(bt + 1) * GPB * 2])

            x1_all = [layer1(bt) for bt in range(NBT)]
            # pre(0) runs during the idle L1 window; pre(bt) for later
            # tiles is emitted between pipelines so it executes while the
            # PREVIOUS tile's matmuls occupy the PE.
            pres = [epilogue_pre(0)]
            for bt in range(NBT):
                batch_tile_pipeline(bt, x1_all[bt], pres[bt])
                if bt + 1 < NBT:
                    pres.append(epilogue_pre(bt + 1))

    nc.compile()
    return nc


def _q8(a):
    import ml_dtypes
    a = np.clip(np.asarray(a, np.float32), -240.0, 240.0)
    return np.ascontiguousarray(a.astype(ml_dtypes.float8_e4m3))


def _pair_pack(W, ncols):
    """[K, ncols] -> [128, npair*2*ncols] fp8 DoubleRow layout:
    out[p, ((j*2)+i)*ncols + n] = W[(2j+i)*128 + p, n]."""
    K = W.shape[0]
    npair = K // 256
    a = np.asarray(W, np.float32).reshape(npair, 2, 128, ncols)
    a = a.transpose(2, 0, 1, 3).reshape(128, npair * 2 * ncols)
    return _q8(a)


def _head_pack(W):
    """[512, 2] -> zero-padded [512, 128] -> DoubleRow pair layout."""
    Wp = np.zeros((W.shape[0], 128), np.float32)
    Wp[:, 0:2] = np.asarray(W, np.float32)
    return _pair_pack(Wp, 128)


def prep_inputs(x, W1, b1, W2, b2, W31, b31, W32, b32,
                W41, b41, W42, b42, W51, b51, W52, b52):
    """Host-side reshapes + fp8 quantization -> per-core in_maps."""
    f32 = np.float32

    shared = {
        "W1": np.ascontiguousarray(np.asarray(W1, f32)),
        "W2p": _pair_pack(W2, D2),
        "W31p": _pair_pack(W31, D3),
        "W32p": _pair_pack(W32, D3),
        "W41p": _pair_pack(W41, D4),
        "W42p": _pair_pack(W42, D4),
        "W51p": _head_pack(W51),
        "W52p": _head_pack(W52),
        "b1p": np.ascontiguousarray(np.asarray(b1, f32).reshape(-1, 128).T),
        "b2p": np.ascontiguousarray(np.asarray(b2, f32).reshape(-1, 128).T),
        "b31p": np.ascontiguousarray(np.asarray(b31, f32).reshape(-1, 128).T),
        "b32p": np.ascontiguousarray(np.asarray(b32, f32).reshape(-1, 128).T),
        "b41p": np.ascontiguousarray(np.asarray(b41, f32).reshape(-1, 128).T),
        "b42p": np.ascontiguousarray(np.asarray(b42, f32).reshape(-1, 128).T),
        "b51p": np.asarray(b51, f32).reshape(2, 1).copy(),
        "b52p": np.asarray(b52, f32).reshape(2, 1).copy(),
    }
    x = np.asarray(x, f32)
    in_maps = []
    for c in range(N_CORES):
        xc = x[c * BC:(c + 1) * BC]
        m = dict(shared)
        m["xT"] = np.ascontiguousarray(xc.T)
        m["Xep"] = np.ascontiguousarray(
            xc.reshape(BC // 32, 32, 5).transpose(1, 0, 2)
            .reshape(32, (BC // 32) * 5))
        in_maps.append(m)
    return in_maps


def unpack_output(results):
    outs = []
    for c in range(N_CORES):
        o = results[c]["out"]  # [32, (BC//32)*2]
        outs.append(o.reshape(32, BC // 32, 2).transpose(1, 0, 2).reshape(BC, 2))
    return np.ascontiguousarray(np.concatenate(outs, axis=0), dtype=np.float32)


_PROG_CACHE = {}


def get_program(consts_key):
    if consts_key not in _PROG_CACHE:
        _PROG_CACHE[consts_key] = build_program(consts_key)
    return _PROG_CACHE[consts_key]


def kernel(x, sgn, mean, std, mean_label, std_label,
           W1, b1, W2, b2, W31, b31, W32, b32,
           W41, b41, W42, b42, W51, b51, W52, b52,
           _trace=False, _tmpdir=None):
    assert int(np.asarray(sgn)) == 1
    consts = (
        tuple(float(v) for v in np.asarray(mean, np.float32)),
        tuple(float(v) for v in np.asarray(std, np.float32)),
        tuple(float(v) for v in np.asarray(mean_label, np.float32)),
        tuple(float(v) for v in np.asarray(std_label, np.float32)),
        tuple(float(v) for v in np.asarray(b51, np.float32)),
        tuple(float(v) for v in np.asarray(b52, np.float32)),
    )
    nc = get_program(consts)
    in_maps = prep_inputs(x, W1, b1, W2, b2, W31, b31, W32, b32,
                          W41, b41, W42, b42, W51, b51, W52, b52)
    res = run_bass_kernel_spmd(nc, in_maps, core_ids=list(range(N_CORES)),
                               trace=_trace, tmpdir=_tmpdir)
    out = unpack_output(res.results)
    kernel.last_result = res
    return out


# revision 34
# speedup vs baseline: 1.0514x; 1.0514x over previous
"""BarrierNet forward pass on 8 Trainium2 NeuronCores (pure data parallel).

Network (per sample, batch 8192 sharded 1024/core):
  x[5] -> 1024 -> 1024 -> {512, 512} -> {512, 512} -> two 2-wide heads
  followed by a closed-form single-constraint QP projection (dCBF barrier).

Layout strategy per core:
  - L1 runs f32r (x precision matters, layer is tiny); all heavy layers
    (L2, L31/32, L41/42, heads) run fp8 e4m3 with DoubleRow perf mode:
    2 contraction rows per PE cell -> half the matmul instructions.
    End-to-end quantization error measured at ~2e-3 norm rel (tol 2e-2).
  - Activations are stored as PAIRED tiles [128, 2, BT] fp8 so each
    DoubleRow matmul consumes k-tiles (2j, 2j+1) from one SBUF tile.
  - A short stream of dummy bf16 matmuls runs while input DMAs land,
    so the PE HAM clock-gate is warm (2.4 GHz) when real work starts.
  - Each 512-sample batch tile runs the full pipeline so the tile-0
    epilogue (DVE/ACT) overlaps tile-1 matmuls on the PE.
  - Head rows land in [32, 512] staging tiles; one DVE 32x32 stream-
    transpose per tile flips them to batch-on-partition.
  - The QP/barrier epilogue runs on [32, 16] strided views (DVE/ACT/
    GpSimd elementwise ops).
  - Host does the cheap reshapes + fp8 weight quantization.
"""

import numpy as np

import concourse.bass as bass
import concourse.tile as tile
from concourse import bacc, mybir
from concourse.bass_utils import run_bass_kernel_spmd

N_CORES = 8
B_FULL = 8192
BC = B_FULL // N_CORES      # batch per core
BT = 512                    # batch tile (matmul moving free dim)
NBT = BC // BT              # batch tiles per core
GPB = BT // 32              # 32-sample groups per batch tile (16)

D1, D2, D3, D4 = 1024, 1024, 512, 512
L1C, L2C, OBS_X, OBS_Y, RADIUS = 3.0, 3.0, 0.0, 7.0, 4.0

N_WARMUP = 4                # dummy PE warmup matmuls (HAM clock-gate)

F32 = mybir.dt.float32
F32R = mybir.dt.float32r
FP8 = mybir.dt.float8e4
BF16 = mybir.dt.bfloat16
AF = mybir.ActivationFunctionType
AL = mybir.AluOpType
DR = mybir.MatmulPerfMode.DoubleRow


def build_program(consts):
    """Build the SPMD Bass program.
    consts = (mean[5], std[5], ml[2], sl[2], b51[2], b52[2])."""
    mean, std, ml, sl, b51v, b52v = consts

    nc = bacc.Bacc("TRN2", target_bir_lowering=False, debug=False,
                   num_devices=N_CORES)

    def din(name, shape, dt):
        return nc.dram_tensor(name, shape, dt, kind="ExternalInput").ap()

    # pair counts per layer (contraction k-tiles / 2)
    P2 = D1 // 256   # 4 pairs into L2
    P3 = D2 // 256   # 4 pairs into L3x
    P4 = D3 // 256   # 2 pairs into L4x
    P5 = D4 // 256   # 2 pairs into heads
    N1, N2, N3, N4 = D1 // 128, D2 // 128, D3 // 128, D4 // 128

    xT_d = din("xT", [5, BC], F32R)
    Xep_d = din("Xep", [32, NBT * GPB * 5], F32)
    W1_d = din("W1", [5, D1], F32R)
    W2_d = din("W2p", [128, P2 * 2 * D2], FP8)
    W31_d = din("W31p", [128, P3 * 2 * D3], FP8)
    W32_d = din("W32p", [128, P3 * 2 * D3], FP8)
    W41_d = din("W41p", [128, P4 * 2 * D4], FP8)
    W42_d = din("W42p", [128, P4 * 2 * D4], FP8)
    # heads zero-padded to M=128 (DoubleRow LDWEIGHTS requires wide M)
    W51_d = din("W51p", [128, P5 * 2 * 128], FP8)
    W52_d = din("W52p", [128, P5 * 2 * 128], FP8)
    b1_d = din("b1p", [128, N1], F32)
    b2_d = din("b2p", [128, N2], F32)
    b31_d = din("b31p", [128, N3], F32)
    b32_d = din("b32p", [128, N3], F32)
    b41_d = din("b41p", [128, N4], F32)
    b42_d = din("b42p", [128, N4], F32)
    b51_d = din("b51p", [2, 1], F32)
    b52_d = din("b52p", [2, 1], F32)
    out_d = nc.dram_tensor("out", [32, NBT * GPB * 2], F32,
                           kind="ExternalOutput").ap()

    with tile.TileContext(nc) as tc:
        with (
            tc.tile_pool(name="wpool", bufs=1) as wp,
            tc.tile_pool(name="acts", bufs=10) as ap_,
            tc.tile_pool(name="misc", bufs=1) as mp,
            tc.tile_pool(name="ep", bufs=1) as ep,
            tc.tile_pool(name="pmm", bufs=8, space="PSUM") as pmm,
        ):
            # ---- PE warmup: dummy bf16 matmuls on a memset tile ---------
            wmt = mp.tile([128, BT], BF16, tag="wmt", name="wmt_t")
            nc.gpsimd.memset(wmt, 0.0)
            wmp = pmm.tile([128, BT], F32, tag="pm", name="wm_ps")
            for i in range(N_WARMUP):
                nc.tensor.matmul(wmp, wmt[:, 0:128], wmt, start=True,
                                 stop=True)

            # ---- input/weight loads -------------------------------------
            # sync ring: L1 inputs first, then weights in need-order
            xT = mp.tile([5, BC], F32R, tag="xT", name="xT_t")
            nc.sync.dma_start(out=xT, in_=xT_d)
            w1 = wp.tile([5, D1], F32R, tag="w1", name="w1_t")
            nc.sync.dma_start(out=w1, in_=W1_d)

            def pair_w(dram, npair, ncols, nm, engine):
                """Load per-pair DoubleRow weight tiles [128, 2, ncols]."""
                ts = []
                for j in range(npair):
                    t = wp.tile([128, 2, ncols], FP8, tag=f"{nm}{j}",
                                name=f"{nm}{j}_t")
                    engine.dma_start(
                        out=t,
                        in_=dram[:, j * 2 * ncols:(j + 1) * 2 * ncols]
                        .rearrange("p (two n) -> p two n", two=2))
                    ts.append(t)
                return ts

            w2 = pair_w(W2_d, P2, D2, "w2", nc.sync)
            w31 = pair_w(W31_d, P3, D3, "w31", nc.sync)
            w32 = pair_w(W32_d, P3, D3, "w32", nc.sync)
            w41 = pair_w(W41_d, P4, D4, "w41", nc.sync)
            w42 = pair_w(W42_d, P4, D4, "w42", nc.sync)

            # gpsimd ring: small late tensors
            def gp_load(dram, shape, tg, dt=F32):
                t = mp.tile(shape, dt, tag=tg, name=f"{tg}_t")
                nc.gpsimd.dma_start(out=t, in_=dram)
                return t

            b1 = gp_load(b1_d, [128, N1], "b1")
            Xep = gp_load(Xep_d, [32, NBT * GPB * 5], "Xep")
            b2 = gp_load(b2_d, [128, N2], "b2")
            b31 = gp_load(b31_d, [128, N3], "b31")
            b32 = gp_load(b32_d, [128, N3], "b32")
            b41 = gp_load(b41_d, [128, N4], "b41")
            b42 = gp_load(b42_d, [128, N4], "b42")
            w51 = pair_w(W51_d, P5, 128, "w51", nc.gpsimd)
            w52 = pair_w(W52_d, P5, 128, "w52", nc.gpsimd)
            b51 = gp_load(b51_d, [2, 1], "b51")
            b52 = gp_load(b52_d, [2, 1], "b52")

            OUT = mp.tile([32, NBT * GPB * 2], F32, tag="OUT", name="OUT_t")

            _cbias_cache = {}

            def cbias(val, parts):
                val = float(val)
                if val not in _cbias_cache:
                    t = ep.tile([128, 1], F32, tag=f"cb{len(_cbias_cache)}",
                                name=f"cb{len(_cbias_cache)}")
                    nc.vector.memset(t, val)
                    _cbias_cache[val] = t
                return _cbias_cache[val][0:parts, :]

            def eact(out, in_, func, bias=0.0, scale=1.0):
                if isinstance(bias, float) and func not in (AF.Copy,):
                    bias = cbias(bias, in_.shape[0])
                nc.scalar.activation(out, in_, func, bias=bias, scale=scale)

            # weighted DVE/ACT alternation for relu+bias. ACT is cheaper
            # per PSUM->SBUF op (172+FD vs 120+FD but 1.2 vs 0.96 GHz) and
            # DVE also owns the epilogue: give ACT 5 of every 8.
            _rb_ctr = [0]

            def relu_bias(t, ps, bias_col):
                c = _rb_ctr[0] % 16
                _rb_ctr[0] += 1
                if c in (0, 2, 4, 7, 9, 11, 13):
                    nc.vector.tensor_scalar(t, ps, bias_col, 0.0,
                                            AL.add, AL.max)
                else:
                    nc.scalar.activation(t, ps, AF.Relu, bias=bias_col)

            HPI = float(np.pi / 2)
            PI = float(np.pi)

            def epilogue_pre(bt):
                """x-only QP/barrier quantities for batch tile bt (no head
                dependence) — runs on DVE/ACT/GpSimd while the PE is still
                in the dense layers."""
                Xv = Xep[:, bt * GPB * 5:(bt + 1) * GPB * 5] \
                    .rearrange("p (f j) -> p f j", j=5)

                def T(nm):
                    return ep.tile([32, GPB], F32, tag=nm, bufs=NBT,
                                   name=f"{nm}_b{bt}")

                def emul(o, a, b):
                    nc.vector.tensor_mul(o, a, b)

                def eadd(o, a, b):
                    nc.vector.tensor_add(o, a, b)

                def stt(o, a, s, op0, b, op1):
                    nc.vector.scalar_tensor_tensor(o, a, float(s), b, op0, op1)

                def gmul(o, a, b):
                    nc.gpsimd.tensor_mul(o, a, b)

                def gadd(o, a, b):
                    nc.gpsimd.tensor_add(o, a, b)

                def gts(o, a, s0, op0):
                    # o = (a op0 s0) + 0.0  (Pool engine lacks stt)
                    nc.gpsimd.tensor_scalar(o, a, float(s0), 0.0, op0, AL.add)

                t1r, w1r = Xv[:, :, 0], Xv[:, :, 1]
                t2r, w2r = Xv[:, :, 2], Xv[:, :, 3]

                if float(std[0]) == 1.0 and float(mean[0]) == 0.0:
                    t1m = t1r
                else:
                    t1m = T("t1m"); eact(t1m, t1r, AF.Copy, bias=float(mean[0]), scale=float(std[0]))
                if float(std[2]) == 1.0 and float(mean[2]) == 0.0:
                    t2m = t2r
                else:
                    t2m = T("t2m"); eact(t2m, t2r, AF.Copy, bias=float(mean[2]), scale=float(std[2]))

                def sincos(theta, nm):
                    ws = T(nm + "_ws"); nc.vector.add_range_wrap(ws, theta, 0.0, PI, 2 * PI)
                    s = T(nm + "_s"); eact(s, ws, AF.Sin)
                    wc = T(nm + "_wc"); nc.vector.add_range_wrap(wc, theta, HPI, PI, 2 * PI)
                    c = T(nm + "_c"); eact(c, wc, AF.Sin)
                    return s, c

                s1, c1 = sincos(t1m, "t1")
                s2, c2 = sincos(t2m, "t2")

                if float(std[1]) == 1.0 and float(mean[1]) == 0.0:
                    w1v = w1r
                else:
                    w1v = T("w1v"); eact(w1v, w1r, AF.Copy, bias=float(mean[1]), scale=float(std[1]))
                if float(std[3]) == 1.0 and float(mean[3]) == 0.0:
                    w2v = w2r
                else:
                    w2v = T("w2v"); eact(w2v, w2r, AF.Copy, bias=float(mean[3]), scale=float(std[3]))

                pxu = T("pxu"); eadd(pxu, c1, c2)
                px = T("px")
                nc.gpsimd.tensor_scalar(px, pxu, L1C, -OBS_X, AL.mult, AL.add)
                pyu = T("pyu"); eadd(pyu, s1, s2)
                py = T("py")
                nc.gpsimd.tensor_scalar(py, pyu, L1C, -OBS_Y, AL.mult, AL.add)

                a1 = T("a1"); emul(a1, s1, w1v)
                a2 = T("a2"); emul(a2, s2, w2v)
                vxn = T("vxn"); eadd(vxn, a1, a2)          # = -vx/3
                bb1 = T("bb1"); emul(bb1, c1, w1v)
                bb2 = T("bb2"); emul(bb2, c2, w2v)
                vyu = T("vyu"); eadd(vyu, bb1, bb2)
                vy = T("vy")
                nc.gpsimd.tensor_scalar(vy, vyu, 3.0, 0.0, AL.mult, AL.add)

                q1 = T("q1"); emul(q1, px, vxn)
                q2 = T("q2"); emul(q2, py, vy)
                bdot2 = T("bdot2"); stt(bdot2, q1, -3.0, AL.mult, q2, AL.add)

                # squares / Lf2b chain on GpSimd (SBUF-only ops)
                w1sq = T("w1sq"); gmul(w1sq, w1v, w1v)
                w2sq = T("w2sq"); gmul(w2sq, w2v, w2v)
                cw1 = T("cw1"); gmul(cw1, c1, w1sq)
                cw2 = T("cw2"); gmul(cw2, c2, w2sq)
                cw = T("cw"); gadd(cw, cw1, cw2)
                sw1 = T("sw1"); gmul(sw1, s1, w1sq)
                sw2 = T("sw2"); gmul(sw2, s2, w2sq)
                sw = T("sw"); gadd(sw, sw1, sw2)
                t1x = T("t1x"); gmul(t1x, px, cw)
                t2y = T("t2y"); gmul(t2y, py, sw)
                txy = T("txy"); gadd(txy, t1x, t2y)
                vv1 = T("vv1"); gmul(vv1, vxn, vxn)
                vv2 = T("vv2"); gmul(vv2, vy, vy)
                vv1s = T("vv1s"); gts(vv1s, vv1, 9.0, AL.mult)
                vv = T("vv"); gadd(vv, vv1s, vv2)
                txys = T("txys"); gts(txys, txy, -3.0, AL.mult)
                Lhalf = T("Lhalf"); gadd(Lhalf, txys, vv)

                g1a = T("g1a"); emul(g1a, px, s1)
                g1b = T("g1b"); emul(g1b, py, c1)
                g2a = T("g2a"); emul(g2a, px, s2)
                g2b = T("g2b"); emul(g2b, py, c2)
                G12 = ep.tile([32, GPB * 2], F32, tag="G12", bufs=NBT,
                              name=f"G12_b{bt}")
                G12v = G12.rearrange("p (f q) -> p f q", q=2)
                G1h, G2h = G12v[:, :, 0], G12v[:, :, 1]
                stt(G1h, g1b, -1.0, AL.mult, g1a, AL.add)  # G1/6
                stt(G2h, g2b, -1.0, AL.mult, g2a, AL.add)  # G2/6

                pxsq = T("pxsq"); gmul(pxsq, px, px)
                pysq = T("pysq"); gmul(pysq, py, py)
                psum2 = T("psum2"); gadd(psum2, pxsq, pysq)
                bar = T("bar"); gts(bar, psum2, -RADIUS * RADIUS, AL.add)

                d1 = T("d1"); emul(d1, G1h, G1h)
                d2 = T("d2"); emul(d2, G2h, G2h)
                den36 = T("den36"); stt(den36, d1, 1e-12 / 36.0, AL.add, d2, AL.add)
                nrec = T("nrec"); nc.vector.reciprocal(nrec, den36)

                return dict(bdot2=bdot2, bar=bar, Lhalf=Lhalf,
                            G1h=G1h, G2h=G2h, G12=G12, nrec=nrec)

            def epilogue_post1(bt, vtb, pre):
                """sigmoid-head-dependent part (can run before head 1)."""
                Yvb = vtb.rearrange("p (f q) -> p f q", q=32)
                sg1, sg2 = Yvb[:, :, 0], Yvb[:, :, 1]

                def T(nm):
                    return ep.tile([32, GPB], F32, tag=nm, bufs=NBT,
                                   name=f"{nm}_b{bt}")

                def emul(o, a, b):
                    nc.vector.tensor_mul(o, a, b)

                def eadd(o, a, b):
                    nc.vector.tensor_add(o, a, b)

                def stt(o, a, s, op0, b, op1):
                    nc.vector.scalar_tensor_tensor(o, a, float(s), b, op0, op1)

                bdot2, bar, Lhalf = pre["bdot2"], pre["bar"], pre["Lhalf"]

                ssum = T("ssum"); eadd(ssum, sg1, sg2)
                sprod = T("sprod"); emul(sprod, sg1, sg2)
                hb = T("hb"); emul(hb, ssum, bdot2)
                hc = T("hc"); emul(hc, sprod, bar)
                va2 = T("va2"); stt(va2, hc, 8.0, AL.mult, Lhalf, AL.add)
                va = T("va"); stt(va, hb, 4.0, AL.mult, va2, AL.add)     # h/2
                return va

            def epilogue_post2(bt, vta, pre, va):
                """head-1-dependent tail of the QP for batch tile bt."""
                Yva = vta.rearrange("p (f q) -> p f q", q=32)
                OUTv = OUT[:, bt * GPB * 2:(bt + 1) * GPB * 2] \
                    .rearrange("p (f i) -> p f i", i=2)

                def T(nm):
                    return ep.tile([32, GPB], F32, tag=nm, bufs=NBT,
                                   name=f"{nm}_b{bt}")

                def emul(o, a, b):
                    nc.vector.tensor_mul(o, a, b)

                def eadd(o, a, b):
                    nc.vector.tensor_add(o, a, b)

                def stt(o, a, s, op0, b, op1):
                    nc.vector.scalar_tensor_tensor(o, a, float(s), b, op0, op1)

                nrec = pre["nrec"]
                G12 = pre["G12"]
                P12 = Yva[:, :, 0:2]  # [32, GPB, 2]

                r12 = ep.tile([32, GPB * 2], F32, tag="r12", bufs=NBT,
                              name=f"r12_b{bt}")
                r12v = r12.rearrange("p (f q) -> p f q", q=2)
                nc.vector.tensor_mul(r12v, G12.rearrange("p (f q) -> p f q", q=2), P12)
                rs = T("rs"); eadd(rs, r12v[:, :, 0], r12v[:, :, 1])
                vb = T("vb"); stt(vb, rs, 3.0, AL.mult, va, AL.add)      # viol=-2vb

                vr = T("vr")
                nc.vector.tensor_scalar(vr, vb, -1.0, 0.0, AL.mult, AL.max)
                lam18 = T("lam18"); emul(lam18, vr, nrec)

                lam18b = bass.AP(tensor=lam18.tensor, offset=lam18.offset,
                                 ap=list(lam18.ap) + [[0, 2]])
                lg12 = ep.tile([32, GPB * 2], F32, tag="lg12", bufs=NBT,
                               name=f"lg12_b{bt}")
                lg12v = lg12.rearrange("p (f q) -> p f q", q=2)
                nc.vector.tensor_mul(
                    lg12v, lam18b, G12.rearrange("p (f q) -> p f q", q=2))
                if (float(sl[0]) == 1.0 and float(sl[1]) == 1.0
                        and float(ml[0]) == 0.0 and float(ml[1]) == 0.0):
                    # out = -(lg12/3 + P12): one DVE op straight into OUT
                    stt(OUTv[:, :, 0:2], lg12v, -1.0 / 3.0, AL.mult, P12,
                        AL.subtract)
                else:
                    u12n = ep.tile([32, GPB * 2], F32, tag="u12n", bufs=NBT,
                                   name=f"u12n_b{bt}")
                    u12v = u12n.rearrange("p (f q) -> p f q", q=2)
                    stt(u12v, lg12v, 1.0 / 3.0, AL.mult, P12, AL.add)
                    eact(OUTv[:, :, 0], u12v[:, :, 0], AF.Copy,
                         bias=-float(ml[0]) / float(sl[0]),
                         scale=-1.0 / float(sl[0]))
                    eact(OUTv[:, :, 1], u12v[:, :, 1], AF.Copy,
                         bias=-float(ml[1]) / float(sl[1]),
                         scale=-1.0 / float(sl[1]))

            def pair_tiles(nm, npair, sfx):
                return [ap_.tile([128, 2, BT], FP8, tag="act",
                                 name=f"{nm}_{j}{sfx}") for j in range(npair)]

            def layer1(bt):
                """L1 (f32r, K=5) for one batch tile -> fp8 pair tiles."""
                sfx = f"b{bt}"
                x1p = pair_tiles("x1", P2, sfx)
                rhs1 = xT[:, bt * BT:(bt + 1) * BT]
                for n in range(N1):
                    ps = pmm.tile([128, BT], F32, tag="pm", name=f"ps1_{n}{sfx}")
                    nc.tensor.matmul(ps, w1[:, n * 128:(n + 1) * 128], rhs1,
                                     start=True, stop=True)
                    relu_bias(x1p[n // 2][:, n % 2, :], ps, b1[:, n:n + 1])
                return x1p

            # head staging tiles memset up-front (GP is idle early)
            x5ab = {}
            for bt in range(NBT):
                x5a = mp.tile([32, BT], F32, tag="x5a", bufs=2,
                              name=f"x5ab{bt}")
                x5b = mp.tile([32, BT], F32, tag="x5b", bufs=2,
                              name=f"x5bb{bt}")
                nc.gpsimd.memset(x5a, 0.0)
                nc.gpsimd.memset(x5b, 0.0)
                x5ab[bt] = (x5a, x5b)

            def dense(bt, nm, xp, ws, bias, n_out, npair, engines=None):
                """fp8 DoubleRow dense layer -> fp8 pair tiles."""
                sfx = f"b{bt}"
                outp = pair_tiles(nm, n_out // 2, sfx)
                for n in range(n_out):
                    ps = pmm.tile([128, BT], F32, tag="pm",
                                  name=f"ps{nm}_{n}{sfx}")
                    for j in range(npair):
                        nc.tensor.matmul(
                            ps, ws[j][:, :, n * 128:(n + 1) * 128], xp[j],
                            start=(j == 0), stop=(j == npair - 1),
                            perf_mode=DR)
                    dst = outp[n // 2][:, n % 2, :]
                    if engines is not None:
                        if engines[n % len(engines)] == "V":
                            nc.vector.tensor_scalar(dst, ps, bias[:, n:n + 1],
                                                    0.0, AL.add, AL.max)
                        else:
                            nc.scalar.activation(dst, ps, AF.Relu,
                                                 bias=bias[:, n:n + 1])
                    else:
                        relu_bias(dst, ps, bias[:, n:n + 1])
                return outp

            def batch_tile_pipeline(bt, x1p, pre):
                """Layers 2+ for one 512-sample batch tile. The sigmoid
                branch is computed first (longest tail chain); cross-engine
                tail work (transpose, post1) is emitted late so dense relus
                aren't stuck behind it in the FIFO engine queues."""
                sfx = f"b{bt}"
                x5a, x5b = x5ab[bt]
                x2 = dense(bt, "x2", x1p, w2, b2, N2, P2)
                x32 = dense(bt, "x32", x2, w32, b32, N3, P3)
                x42 = dense(bt, "x42", x32, w42, b42, N4, P4, engines="VAVA")
                x31 = dense(bt, "x31", x2, w31, b31, N3, P3)
                ph2 = pmm.tile([128, BT], F32, tag="pm", name=f"ph2{sfx}")
                for j in range(P5):
                    nc.tensor.matmul(ph2, w52[j], x42[j],
                                     start=(j == 0), stop=(j == P5 - 1),
                                     perf_mode=DR)
                nc.scalar.activation(x5b[0:2, :], ph2[0:2, :], AF.Sigmoid,
                                     bias=b52)
                x41 = dense(bt, "x41", x31, w41, b41, N4, P4, engines="VAVA")
                vtb = mp.tile([32, BT], F32, tag="vtb", bufs=2, name=f"vtb{sfx}")
                nc.vector.transpose(vtb, x5b)
                va = epilogue_post1(bt, vtb, pre)
                ph1 = pmm.tile([128, BT], F32, tag="pm", name=f"ph1{sfx}")
                for j in range(P5):
                    nc.tensor.matmul(ph1, w51[j], x41[j],
                                     start=(j == 0), stop=(j == P5 - 1),
                                     perf_mode=DR)
                nc.scalar.activation(x5a[0:2, :], ph1[0:2, :], AF.Identity,
                                     bias=b51)
                vta = mp.tile([32, BT], F32, tag="vta", bufs=2, name=f"vta{sfx}")
                nc.vector.transpose(vta, x5a)

                epilogue_post2(bt, vta, pre, va)
                nc.sync.dma_start(
                    out=out_d[:, bt * GPB * 2:(bt + 1) * GPB * 2],
                    in_=OUT[:, bt * GPB * 2:(bt + 1) * GPB * 2])

            x1_all = [layer1(bt) for bt in range(NBT)]
            # pre(0) runs during the idle L1 window; pre(bt) for later
            # tiles is emitted between pipelines so it executes while the
            # PREVIOUS tile's matmuls occupy the PE.
            pres = [epilogue_pre(0)]
            for bt in range(NBT):
                batch_tile_pipeline(bt, x1_all[bt], pres[bt])
                if bt + 1 < NBT:
                    pres.append(epilogue_pre(bt + 1))

    nc.compile()
    return nc


def _q8(a):
    import ml_dtypes
    a = np.clip(np.asarray(a, np.float32), -240.0, 240.0)
    return np.ascontiguousarray(a.astype(ml_dtypes.float8_e4m3))


def _pair_pack(W, ncols):
    """[K, ncols] -> [128, npair*2*ncols] fp8 DoubleRow layout:
    out[p, ((j*2)+i)*ncols + n] = W[(2j+i)*128 + p, n]."""
    K = W.shape[0]
    npair = K // 256
    a = np.asarray(W, np.float32).reshape(npair, 2, 128, ncols)
    a = a.transpose(2, 0, 1, 3).reshape(128, npair * 2 * ncols)
    return _q8(a)


def _head_pack(W):
    """[512, 2] -> zero-padded [512, 128] -> DoubleRow pair layout."""
    Wp = np.zeros((W.shape[0], 128), np.float32)
    Wp[:, 0:2] = np.asarray(W, np.float32)
    return _pair_pack(Wp, 128)


def prep_inputs(x, W1, b1, W2, b2, W31, b31, W32, b32,
                W41, b41, W42, b42, W51, b51, W52, b52):
    """Host-side reshapes + fp8 quantization -> per-core in_maps."""
    f32 = np.float32

    shared = {
        "W1": np.ascontiguousarray(np.asarray(W1, f32)),
        "W2p": _pair_pack(W2, D2),
        "W31p": _pair_pack(W31, D3),
        "W32p": _pair_pack(W32, D3),
        "W41p": _pair_pack(W41, D4),
        "W42p": _pair_pack(W42, D4),
        "W51p": _head_pack(W51),
        "W52p": _head_pack(W52),
        "b1p": np.ascontiguousarray(np.asarray(b1, f32).reshape(-1, 128).T),
        "b2p": np.ascontiguousarray(np.asarray(b2, f32).reshape(-1, 128).T),
        "b31p": np.ascontiguousarray(np.asarray(b31, f32).reshape(-1, 128).T),
        "b32p": np.ascontiguousarray(np.asarray(b32, f32).reshape(-1, 128).T),
        "b41p": np.ascontiguousarray(np.asarray(b41, f32).reshape(-1, 128).T),
        "b42p": np.ascontiguousarray(np.asarray(b42, f32).reshape(-1, 128).T),
        "b51p": np.asarray(b51, f32).reshape(2, 1).copy(),
        "b52p": np.asarray(b52, f32).reshape(2, 1).copy(),
    }
    x = np.asarray(x, f32)
    in_maps = []
    for c in range(N_CORES):
        xc = x[c * BC:(c + 1) * BC]
        m = dict(shared)
        m["xT"] = np.ascontiguousarray(xc.T)
        m["Xep"] = np.ascontiguousarray(
            xc.reshape(BC // 32, 32, 5).transpose(1, 0, 2)
            .reshape(32, (BC // 32) * 5))
        in_maps.append(m)
    return in_maps


def unpack_output(results):
    outs = []
    for c in range(N_CORES):
        o = results[c]["out"]  # [32, (BC//32)*2]
        outs.append(o.reshape(32, BC // 32, 2).transpose(1, 0, 2).reshape(BC, 2))
    return np.ascontiguousarray(np.concatenate(outs, axis=0), dtype=np.float32)


_PROG_CACHE = {}


def get_program(consts_key):
    if consts_key not in _PROG_CACHE:
        _PROG_CACHE[consts_key] = build_program(consts_key)
    return _PROG_CACHE[consts_key]


def kernel(x, sgn, mean, std, mean_label, std_label,
           W1, b1, W2, b2, W31, b31, W32, b32,
           W41, b41, W42, b42, W51, b51, W52, b52,
           _trace=False, _tmpdir=None):
    assert int(np.asarray(sgn)) == 1
    consts = (
        tuple(float(v) for v in np.asarray(mean, np.float32)),
        tuple(float(v) for v in np.asarray(std, np.float32)),
        tuple(float(v) for v in np.asarray(mean_label, np.float32)),
        tuple(float(v) for v in np.asarray(std_label, np.float32)),
        tuple(float(v) for v in np.asarray(b51, np.float32)),
        tuple(float(v) for v in np.asarray(b52, np.float32)),
    )
    nc = get_program(consts)
    in_maps = prep_inputs(x, W1, b1, W2, b2, W31, b31, W32, b32,
                          W41, b41, W42, b42, W51, b51, W52, b52)
    res = run_bass_kernel_spmd(nc, in_maps, core_ids=list(range(N_CORES)),
                               trace=_trace, tmpdir=_tmpdir)
    out = unpack_output(res.results)
    kernel.last_result = res
    return out
